# revision 9
# baseline (speedup 1.0000x reference)
"""Trainium2 Bass kernel for the per-task embedding MLP (embedding_lookup).

Computation (per sample j with task t = task_ids[j]):
    h      = x[j] @ l1_emb[t].reshape(256, 128) + l1_bias[t]
    g      = gelu_exact(h)
    out[j] = sum(g * l2_emb[t]) + l2_bias[t, 0]

Strategy: shard the *task* axis across the 8 cores (125 contiguous tasks per
core), so each core streams its slab of l1_emb exactly once (the memory
roofline).  Samples are routed (host-side index math only) to the core
owning their task and packed into a slot grid of W=8 columns per group
(tasks with more than W samples get extra groups with duplicated weight
rows; tasks with no samples get none), so all 8 cores run one identical
SPMD program: per group, two K=128 matmuls of the task's [256,128] fp8
weights against its [256,W] fp16 x-columns accumulate hT[128, cols] in
PSUM; a per-block "bias matmul" (block's l1_bias rows against a one-hot
group->slot indicator) accumulates b1 into the same PSUM, so the epilogue
is just gelu (ACT from PSUM, fp16 out) -> *w2 (DVE, 16-bit) -> hidden-dim
reduce via a ones-vector matmul -> +b2 -> DMA out.

Precision: w1 is host-cast to fp8 E3M4 (float8e3) — halves the dominant
l1_emb DMA stream vs fp16; x stays fp16 (mixed-dtype matmuls are fine).
b1/w2 ride in fp16, b2 in fp32, PSUM accumulation in fp32.  Measured
end-to-end L2 relative error ~1.3e-2 (dominated by the 4-bit-mantissa
weight quantization), within the 2e-2 gate.

DMA plan: a dma_start costs ~0.6us of issuing-engine time, so descriptors
are few and large.  The sync HWDGE ring carries w1 block slabs interleaved
with the x slices that upcoming blocks need (FIFO order = natural pacing);
small lead-in blocks (2/3/10/15 groups) are single descriptors so the
stream saturates immediately and the first matmul starts as early as the
DGE cold latency allows.  The scalar HWDGE ring carries the packed fp16
constants (indicator + per-block b1 + w2T) and b2 in parallel.  Mid-kernel
output writebacks go on the otherwise-idle gpsimd SWDGE so they never
stall the sync ring's descriptor issue; the final tail output goes on sync
once w1 issue is done.
"""

import numpy as np
import ml_dtypes

import concourse.bacc as bacc
import concourse.mybir as mybir
import concourse.tile as tile
from concourse.bass_utils import run_bass_kernel_spmd

NUM_TASKS = 1000
N_FEATURES = 256
HIDDEN = 128
BATCH = 4096
N_CORES = 8
TPC = NUM_TASKS // N_CORES  # tasks per core = 125

# Module-level knobs for the test harness (the grader just calls kernel()).
W1_DTYPE = "float8e3"  # w1 slab dtype: "float8e3" | "float16"
TRACE = False
TMPDIR = None  # optional fixed artifact dir for profiling runs
SIM_CORES = None  # e.g. [0]: run CoreSim for those cores instead of hardware
SIM_EXECUTOR_CLS = None  # optional InstructionExecutor subclass for CoreSim
LAST_RESULTS = None

_PROGRAM_CACHE = {}

W = 8           # sample slots per group
GB = 60         # max groups per PSUM block (GB*W*4B <= 2KB bank)
HEAD = [5, 15]  # lead-in block sizes (one DMA descriptor each)
TAIL = [10, 5]  # ramp-down block sizes (short post-last-matmul chain)
MAXG = 30       # max groups per w1 DMA descriptor in big blocks
HB = 20         # head groups (blocks 0-1) whose b1T rides in cstW's tail


def _np_dtype(name):
    return {
        "float8e3": ml_dtypes.float8_e3m4,
        "float16": np.float16,
        "float32": np.float32,
    }[name]


def _block_sizes(NG):
    rem = NG - sum(HEAD) - sum(TAIL)
    assert rem > 0
    sizes = HEAD + [GB] * (rem // GB) + ([rem % GB] if rem % GB else []) + TAIL
    assert sum(sizes) == NG and all(s <= GB for s in sizes)
    return sizes


def _x_chunks(sizes):
    """x is interleaved on the sync ring: chunk i covers a run of blocks and
    is enqueued just before those blocks' w1 slabs.  Returns, per sync-ring
    slot, either ('x', lo_col, hi_col) or ('w1', block_idx)."""
    # runs of blocks per x chunk: [0,1], [2,3], [4], [5..]
    runs = [[0, 1], [2, 3], [4], list(range(5, len(sizes)))]
    runs = [[b for b in r if b < len(sizes)] for r in runs]
    runs = [r for r in runs if r]
    plan = []
    for r in runs:
        lo = sum(sizes[:r[0]]) * W
        hi = sum(sizes[:r[-1] + 1]) * W
        plan.append(("x", lo, hi))
        for b in r:
            plan.append(("w1", b))
    return plan


def _build_program(NG, w1_dtype):
    sizes = _block_sizes(NG)
    NB = len(sizes)
    NSLOT = NG * W
    f32 = mybir.dt.float32
    f16 = mybir.dt.float16
    wdt = getattr(mybir.dt, w1_dtype)

    IND = GB * W                      # indicator columns in cstA
    CCA = IND + NB * HIDDEN           # cstA fp16 columns
    b1off = lambda b: IND + b * HIDDEN

    nc = bacc.Bacc("TRN2", target_bir_lowering=False, debug=False)

    xT_d = nc.dram_tensor("xT", [2, 128, NSLOT], f16, kind="ExternalInput").ap()
    w1_d = nc.dram_tensor(
        "w1s", [NG * N_FEATURES * HIDDEN], wdt, kind="ExternalInput"
    ).ap()
    cstA_d = nc.dram_tensor("cstA", [GB, CCA], f16, kind="ExternalInput").ap()
    cstW_d = nc.dram_tensor("cstW", [128, NG + HB], f16, kind="ExternalInput").ap()
    b2_d = nc.dram_tensor("b2r", [1, NG], f32, kind="ExternalInput").ap()
    out_d = nc.dram_tensor("out", [1, NSLOT], f32, kind="ExternalOutput").ap()

    gelu = mybir.ActivationFunctionType.Gelu

    with tile.TileContext(nc) as tc:
        with (
            tc.tile_pool(name="const", bufs=1) as constp,
            tc.tile_pool(name="w1pool", bufs=5) as w1p,
            tc.tile_pool(name="work", bufs=3) as workp,
            tc.tile_pool(name="hpsum", bufs=6, space="PSUM") as hpsp,
            tc.tile_pool(name="opsum", bufs=2, space="PSUM") as opsp,
        ):
            xc0 = constp.tile([128, NSLOT], f16)
            xc1 = constp.tile([128, NSLOT], f16)
            cstA = constp.tile([GB, CCA], f16)
            cstW = constp.tile([128, NG + HB], f16)
            b2r = constp.tile([1, NG], f32)
            # scalar ring, in parallel with sync's w1 stream: x lead slices
            # (blocks 0..3) first so the first matmuls only gate on w1, then
            # the packed constants, then the x bulk (needed from block 4,
            # ~12.5us, well after its descriptor lands)
            xlc = sum(sizes[:2]) * W
            xmc = sum(sizes[:3]) * W
            nc.scalar.dma_start(out=xc0[:, :xlc], in_=xT_d[0][:, :xlc])
            nc.scalar.dma_start(out=xc1[:, :xlc], in_=xT_d[1][:, :xlc])
            nc.scalar.dma_start(out=cstW, in_=cstW_d)
            nc.scalar.dma_start(out=b2r, in_=b2_d)
            nc.scalar.dma_start(out=cstA, in_=cstA_d)
            nc.scalar.dma_start(out=xc0[:, xlc:xmc], in_=xT_d[0][:, xlc:xmc])
            nc.scalar.dma_start(out=xc1[:, xlc:xmc], in_=xT_d[1][:, xlc:xmc])
            nc.scalar.dma_start(out=xc0[:, xmc:], in_=xT_d[0][:, xmc:])
            nc.scalar.dma_start(out=xc1[:, xmc:], in_=xT_d[1][:, xmc:])

            cones = constp.tile([128, 1], f16)
            nc.vector.memset(cones, 1.0)

            out_sb = constp.tile([1, NSLOT], f32)

            # sync ring: w1 block slabs only (plus the final writeback)
            w1tiles = {}
            w1off = 0
            for b, gbt in enumerate(sizes):
                ln = 128 * gbt * 2 * 128
                w1t = w1p.tile([128, gbt, 2, 128], wdt, tag="w1t")
                blk = w1_d[w1off:w1off + ln].rearrange(
                    "(p g c h) -> p g c h", p=128, g=gbt, c=2
                )
                q0 = 0
                while q0 < gbt:
                    q1 = min(q0 + MAXG, gbt)
                    nc.sync.dma_start(out=w1t[:, q0:q1], in_=blk[:, q0:q1])
                    q0 = q1
                w1tiles[b] = w1t
                w1off += ln

            # the hidden-dim reduce for block k is issued after block k+1's
            # w1 matmuls: the PE queue is strict FIFO, so an in-line reduce
            # (which waits on the scalar/vector gelu+mul chain) would stall
            # the next block's matmuls by ~1-2.5us per block
            pend = []

            def _finish(item):
                pb, pg0, pgbt, pprod = item
                pcols = pgbt * W
                pcsl = slice(pg0 * W, pg0 * W + pcols)
                ops = opsp.tile([1, pcols], f32, tag="ops")
                nc.tensor.matmul(ops, lhsT=cones, rhs=pprod, start=True, stop=True)
                b2v = b2r[:, pg0:pg0 + pgbt].unsqueeze(2).broadcast_to([1, pgbt, W])
                nc.vector.tensor_add(
                    out_sb[:, pcsl].rearrange("p (g w) -> p g w", w=W),
                    ops.rearrange("p (g w) -> p g w", w=W),
                    b2v,
                )
                if pb < NB - 2:
                    nc.gpsimd.dma_start(out=out_d[:, pcsl], in_=out_sb[:, pcsl])

            from contextlib import ExitStack as _ES

            for b, gbt in enumerate(sizes):
                _g = _ES(); _g.enter_context(tc.tile_wait_until(b + 1))
                g0 = sum(sizes[:b])
                cols = gbt * W
                base = g0 * W
                csl = slice(base, base + cols)
                w1t = w1tiles[b]

                ps = hpsp.tile([128, cols], f32, tag="hps")
                if b >= 2:
                    # b1 lands in PSUM first (start=True clears the whole
                    # zero region, so the block-wide write must precede the
                    # per-group accumulations): b1blk.T @ one-hot
                    nc.tensor.matmul(
                        ps,
                        lhsT=cstA[0:gbt, b1off(b):b1off(b) + HIDDEN],
                        rhs=cstA[0:gbt, 0:cols],
                        start=True, stop=False, skip_group_check=True,
                    )
                for jj in range(gbt):
                    sl = slice(jj * W, (jj + 1) * W)
                    xsl = slice(base + jj * W, base + (jj + 1) * W)
                    first = b < 2
                    last = b >= 2 and jj == gbt - 1
                    nc.tensor.matmul(
                        ps[:, sl], lhsT=w1t[:, jj, 0], rhs=xc0[:, xsl],
                        start=first, stop=False, skip_group_check=True,
                    )
                    nc.tensor.matmul(
                        ps[:, sl], lhsT=w1t[:, jj, 1], rhs=xc1[:, xsl],
                        start=False, stop=first or last, skip_group_check=True,
                    )

                # epilogue: gelu straight off PSUM (fp16 out), *w2 on DVE
                # (16-bit, 2x rate), ones-matmul hidden reduce, +b2.  The
                # two head blocks run before cstA lands, so their b1 is
                # added by a small STT (b1T rides in cstW's tail columns).
                esb = workp.tile([128, cols], f16, tag="esb")
                prodt = workp.tile([128, cols], f16, tag="prodt")
                halves = [(0, gbt // 2), (gbt // 2, gbt)] if gbt > 15 else [(0, gbt)]
                for ga, gz in halves:
                    hsl = slice(ga * W, gz * W)
                    n_g = gz - ga
                    if b < 2:
                        hsf = workp.tile([128, cols], f32, tag="hsf")
                        b1v = (
                            cstW[:, NG + g0 + ga:NG + g0 + gz]
                            .unsqueeze(2).broadcast_to([128, n_g, W])
                        )
                        nc.vector.scalar_tensor_tensor(
                            hsf[:, hsl].rearrange("p (g w) -> p g w", w=W),
                            ps[:, hsl].rearrange("p (g w) -> p g w", w=W),
                            1.0, b1v,
                            op0=mybir.AluOpType.mult, op1=mybir.AluOpType.add,
                        )
                        nc.scalar.activation(esb[:, hsl], hsf[:, hsl], gelu)
                    else:
                        nc.scalar.activation(esb[:, hsl], ps[:, hsl], gelu)
                    w2v = (
                        cstW[:, g0 + ga:g0 + gz]
                        .unsqueeze(2).broadcast_to([128, n_g, W])
                    )
                    nc.vector.tensor_mul(
                        prodt[:, hsl].rearrange("p (g w) -> p g w", w=W),
                        esb[:, hsl].rearrange("p (g w) -> p g w", w=W),
                        w2v,
                    )
                if pend:
                    _finish(pend.pop(0))
                pend.append((b, g0, gbt, prodt))
                _g.close()

            with tc.tile_wait_until(len(sizes) + 1):
                _finish(pend.pop(0))
                tb = sum(sizes[:-2]) * W
                nc.sync.dma_start(out=out_d[:, tb:], in_=out_sb[:, tb:])

    nc.compile()
    return nc


def _get_program(NG, w1_dtype):
    key = (NG, w1_dtype)
    if key not in _PROGRAM_CACHE:
        _PROGRAM_CACHE[key] = _build_program(NG, w1_dtype)
    return _PROGRAM_CACHE[key]


def kernel(x, task_ids, l1_emb, l1_bias, l2_emb, l2_bias):
    global LAST_RESULTS
    x = np.ascontiguousarray(np.asarray(x, dtype=np.float32))
    tid = np.asarray(task_ids).astype(np.int64)
    l1_emb = np.ascontiguousarray(np.asarray(l1_emb, dtype=np.float32))
    l1_bias = np.ascontiguousarray(np.asarray(l1_bias, dtype=np.float32))
    l2_emb = np.ascontiguousarray(np.asarray(l2_emb, dtype=np.float32))
    l2_bias = np.ascontiguousarray(np.asarray(l2_bias, dtype=np.float32))

    B = x.shape[0]
    assert x.shape == (BATCH, N_FEATURES) and tid.shape == (BATCH,)

    wdt = _np_dtype(W1_DTYPE)

    # A "group" is (task, slice of up to W of its samples).  Tasks with more
    # than W samples get several groups; tasks with no samples get none.
    counts = np.bincount(tid, minlength=NUM_TASKS)
    ngroups = (-(-counts // W)).astype(np.int64)  # ceil, 0 for empty tasks
    ng_core = ngroups.reshape(N_CORES, TPC).sum(axis=1)
    NG = -(-int(ng_core.max()) // 5) * 5  # round up to a multiple of 5
    NSLOT = NG * W

    # within-core group base of each task
    gbase = np.empty(NUM_TASKS, dtype=np.int64)
    for c in range(N_CORES):
        sl = slice(c * TPC, (c + 1) * TPC)
        cs = np.cumsum(ngroups[sl])
        gbase[sl] = cs - ngroups[sl]

    # slot routing: sample j -> (core, slot)
    order = np.argsort(tid, kind="stable")
    sorted_tid = tid[order]
    starts = np.flatnonzero(np.r_[True, np.diff(sorted_tid) != 0])
    run_len = np.diff(np.r_[starts, B])
    run_pos = np.arange(B) - np.repeat(starts, run_len)
    occ = np.empty(B, dtype=np.int64)
    occ[order] = run_pos
    core = tid // TPC
    slot = (gbase[tid] + occ // W) * W + occ % W

    # scatter x into per-core transposed, padded slot grids
    xT = np.zeros((N_CORES, N_FEATURES, NSLOT), dtype=np.float16)
    xT[core, :, slot] = x.astype(np.float16)

    sizes = _block_sizes(NG)
    NB = len(sizes)
    IND = GB * W
    CCA = IND + NB * HIDDEN

    # indicator: ind[g, col] = 1.0 where col // W == g
    ind = np.zeros((GB, IND), dtype=np.float16)
    ind[np.arange(IND) // W, np.arange(IND)] = 1.0

    in_maps = []
    for c in range(N_CORES):
        t0 = c * TPC
        sl = slice(t0, t0 + TPC)
        # task id of each group (padded to NG with the core's first task)
        gtask = np.repeat(np.arange(t0, t0 + TPC), ngroups[sl])
        if len(gtask) < NG:
            gtask = np.r_[gtask, np.full(NG - len(gtask), t0)]
        rows = l1_emb[gtask]  # [NG, 32768]
        cstA = np.zeros((GB, CCA), dtype=np.float16)
        cstA[:, :IND] = ind
        # pack w1 per block: [gbt, 2, 128, 128] -> [128, gbt, 2, 128] flat
        parts = []
        cum = 0
        for b, gbt in enumerate(sizes):
            blk = rows[cum:cum + gbt]
            blk = blk.reshape(gbt, 2, 128, 128).transpose(2, 0, 1, 3)
            parts.append(blk.astype(wdt).reshape(-1))
            cstA[0:gbt, IND + b * HIDDEN:IND + (b + 1) * HIDDEN] = (
                l1_bias[gtask[cum:cum + gbt]]
            )
            cum += gbt
        in_maps.append({
            "xT": np.ascontiguousarray(xT[c].reshape(2, 128, NSLOT)),
            "w1s": np.concatenate(parts),
            "cstA": cstA,
            "cstW": np.ascontiguousarray(np.concatenate(
                [l2_emb[gtask].T, l1_bias[gtask[:HB]].T], axis=1
            ).astype(np.float16)),
            "b2r": np.ascontiguousarray(l2_bias[gtask].reshape(1, NG)),
        })

    nc = _get_program(NG, W1_DTYPE)
    if SIM_CORES is not None:
        from concourse.bass_interp import CoreSim

        sim_results = []
        for c in range(N_CORES):
            if c in SIM_CORES:
                kw = {}
                if SIM_EXECUTOR_CLS is not None:
                    kw["executor_cls"] = SIM_EXECUTOR_CLS
                sim = CoreSim(nc, publish_trace=False, **kw)
                for k, v in in_maps[c].items():
                    sim.tensor(k)[:] = v
                sim.simulate()
                sim_results.append({"out": np.array(sim.tensor("out"))})
            else:
                sim_results.append({"out": np.zeros((1, NSLOT), np.float32)})
        outs = np.stack([r["out"].reshape(NSLOT) for r in sim_results])
        logits = outs[core, slot]
        return logits[:, None].astype(np.float32)

    res = run_bass_kernel_spmd(
        nc, in_maps, core_ids=list(range(N_CORES)), trace=TRACE, tmpdir=TMPDIR,
    )
    LAST_RESULTS = res

    outs = np.stack([r["out"].reshape(NSLOT) for r in res.results])
    logits = outs[core, slot]
    return logits[:, None].astype(np.float32)
